# revision 58
# baseline (speedup 1.0000x reference)
"""AutoMTLSuperNet (moe_routing) Trainium2 kernel.

Batch data-parallel over 8 NeuronCores (2048 samples each, params
replicated). On-chip layout is output-channel-major ([oc, batch]); all
matmuls bf16 with f32 PSUM accumulation.

v4: batch processed as 2 pair-chunks of 1024 columns (elementwise/ACT ops
run on [*,1024] tiles to amortize per-op overhead; matmuls still stream
512-col halves into [128,1024] psum tiles). The relu candidate weight is
folded into the W_c0 matmul weights on host so the relu branch fuses into
one scalar_tensor_tensor (relu + add). Tanh-branch scale-mults run on the
otherwise idle GpSimd engine. Gate softmax normalization is folded into
the broadcast rows (no per-n reciprocal multiply). Final domain mix is
expert-major with PE row-broadcasts, then transposed once.
"""

import numpy as np
import ml_dtypes

import concourse.bass as bass
import concourse.bacc as bacc
import concourse.mybir as mybir
import concourse.tile as tile
from concourse.bass_utils import run_bass_kernel_spmd

# ---- problem dims (hardcoded per contract) ----
B, F, E, D = 16384, 26, 16, 13
NE, ND, NC = 4, 3, 3
GIN = E * (F + 1) + D            # 445
H, OUT = 256, 128
N_CORES = 8
B_LOC = B // N_CORES             # 2048
SBC = 512                        # matmul / phase0 sub-chunk columns
NPC = 1024                       # pair-chunk columns for elementwise ops
NCHUNK = B_LOC // NPC            # 2 pair-chunks
NSUB = B_LOC // SBC              # 4 sub-chunks
KSP = F * E                      # 416 flattened sparse dim
KPAD = 448
BF16 = mybir.dt.bfloat16
F32 = mybir.dt.float32

AF = mybir.ActivationFunctionType
ALU = mybir.AluOpType


def _bf16(x):
    return np.asarray(x, dtype=ml_dtypes.bfloat16)


def _softmax_np(a):
    a = np.asarray(a, dtype=np.float64)
    m = a.max(axis=-1, keepdims=True)
    e = np.exp(a - m)
    return (e / e.sum(axis=-1, keepdims=True)).astype(np.float32)


def prep_shared(inputs):
    """Host prep of all parameter tensors (input-layout + parameter-only math)."""
    f32 = np.float32
    gate_w = 1.0 / (1.0 + np.exp(-inputs['feat_alpha'].astype(np.float64)))  # [NE,F]
    gate_w = gate_w.astype(f32)

    W_l0b0 = inputs['W_l0b0'].astype(f32)   # [NE,NC,GIN,H]
    W_l0b1 = inputs['W_l0b1'].astype(f32)   # [NE,NC,H,OUT]
    W_l1b0 = inputs['W_l1b0'].astype(f32)   # [NE,NC,OUT,H]
    W_l1b1 = inputs['W_l1b1'].astype(f32)   # [NE,NC,H,OUT]

    # candidate softmax weights per mixed-op layer: [4][NE,NC]
    wmix_l = [_softmax_np(inputs[k]) for k in ('a_l0b0', 'a_l0b1', 'a_l1b0', 'a_l1b1')]

    # fold the c=0 (relu) candidate weight into the matmul weights: since
    # w0>0, w0*relu(p+b) = relu(w0*p + w0*b). Bias tables scale c=0 rows too.
    W_l0b0 = W_l0b0.copy()
    W_l0b1 = W_l0b1.copy()
    W_l1b0 = W_l1b0.copy()
    W_l1b1 = W_l1b1.copy()
    for n in range(NE):
        W_l0b0[n, 0] *= wmix_l[0][n, 0]
        W_l0b1[n, 0] *= wmix_l[1][n, 0]
        W_l1b0[n, 0] *= wmix_l[2][n, 0]
        W_l1b1[n, 0] *= wmix_l[3][n, 0]

    # ---- Wl0: lhsT ktiles [4,128, 3072]; col = n*768 + c*256 + h ----
    Wl0 = np.zeros((4, 128, NE * NC * H), dtype=f32)
    Wsp = np.zeros((KSP, NE, NC, H), dtype=f32)
    for n in range(NE):
        gvec = np.repeat(gate_w[n], E)                      # [416]
        Wsp[:, n] = W_l0b0[n, :, :KSP, :].transpose(1, 0, 2) * gvec[:, None, None]
    Wsp = Wsp.reshape(KSP, NE * NC * H)
    for kt in range(3):
        Wl0[kt, :, :] = Wsp[kt * 128:(kt + 1) * 128]
    # kt3 rows: [0:32]=sparse 384..415, [32:45]=dense, [45:64]=0, [64:128]=fm
    Wl0[3, 0:32, :] = Wsp[384:416]
    for d in range(D):
        Wl0[3, 32 + d, :] = W_l0b0[:, :, KSP + E + d, :].reshape(-1)
    for n in range(NE):
        for e in range(E):
            Wl0[3, 64 + n * 16 + e, n * 768:(n + 1) * 768] = \
                W_l0b0[n, :, KSP + e, :].reshape(768)

    # ---- GsWg: [4,128,108]; 0:64 Gs (n*16+e), 64:80 g0 (e*4+n), 96:108 g1 ----
    GsWg = np.zeros((4, 128, 108), dtype=f32)
    Gq = np.zeros((4, 128, 64), dtype=f32)
    for fe in range(KSP):
        kt, i = divmod(fe, 128)
        f_, e_ = divmod(fe, E)
        for n in range(NE):
            g = gate_w[n, f_]
            GsWg[kt, i, n * 16 + e_] = g
            Gq[kt, i, n * 16 + e_] = 0.5 * g * g   # 0.5 pre-folded
    Wg0, Wg1 = inputs['Wg0'].astype(f32), inputs['Wg1'].astype(f32)
    for i in range(KSP):
        kt, r = divmod(i, 128)
        for n in range(NE):
            for e in range(NE):
                GsWg[kt, r, 64 + e * 4 + n] = Wg0[n, i, e]
        for d in range(ND):
            for e in range(NE):
                GsWg[kt, r, 96 + d * 4 + e] = Wg1[d, i, e]
    gbias = np.zeros((44, 1), dtype=f32)
    for n in range(NE):
        for e in range(NE):
            gbias[e * 4 + n, 0] = inputs['bg0'][n, e] + inputs['beta0'][n, e]
    for d in range(ND):
        for e in range(NE):
            gbias[32 + d * 4 + e, 0] = inputs['bg1'][d, e] + inputs['beta1'][d, e]
    # sel16 [16,4]: row e*4+n -> col n  (row sums per layer-0 gate n)
    sel16 = np.zeros((16, 4), dtype=f32)
    for e in range(NE):
        for n in range(NE):
            sel16[e * 4 + n, n] = 1.0
    # r16sel [4,16]: broadcast r0 row n to rows e*4+n
    r16sel = np.zeros((4, 16), dtype=f32)
    for e in range(NE):
        for n in range(NE):
            r16sel[n, e * 4 + n] = 1.0
    # selbc/selbr: lhsT picks one row of rhs, broadcast to 128 partitions
    selbc = np.zeros((16, 16, 128), dtype=f32)
    for r_ in range(16):
        selbc[r_, r_, :] = 1.0
    selbr = np.zeros((4, 4, 128), dtype=f32)
    for r_ in range(4):
        selbr[r_, r_, :] = 1.0
    # oh3 [3,12]: broadcast domain-onehot row d to rows 4d..4d+3
    oh3 = np.zeros((3, 12), dtype=f32)
    for d in range(ND):
        oh3[d, 4 * d:4 * d + 4] = 1.0
    # sel12 [12,36]: cols 0:4 = all-ones (expert sum); cols 32:36 pick expert e
    sel12 = np.zeros((12, 36), dtype=f32)
    sel12[:, 0:4] = 1.0
    for d in range(ND):
        for e in range(NE):
            sel12[4 * d + e, 32 + e] = 1.0

    # ---- later layer weights ----
    Wb1 = np.zeros((NE, H, NC * OUT), dtype=f32)       # lhsT col = c*128+o
    for n in range(NE):
        Wb1[n] = W_l0b1[n].transpose(1, 0, 2).reshape(H, NC * OUT)
    W10 = np.zeros((NE, OUT, NC * H), dtype=f32)       # col = c*256+h
    for n in range(NE):
        W10[n] = W_l1b0[n].transpose(1, 0, 2).reshape(OUT, NC * H)
    W11 = np.zeros((NE, H, NC * OUT), dtype=f32)
    for n in range(NE):
        W11[n] = W_l1b1[n].transpose(1, 0, 2).reshape(H, NC * OUT)

    # ---- bias column tables (per-partition vectors), w-scaled for relu c=0 ----
    def bias_table(bmat, wl, n_mt):
        Wd = bmat.shape[-1]
        tbl = np.zeros((128, NE * NC * (Wd // 128)), dtype=f32)
        m = 0
        for n in range(NE):
            for c in range(NC):
                for hh in range(Wd // 128):
                    v = bmat[n, c, hh * 128:(hh + 1) * 128].astype(f32)
                    if c == 0:
                        v = v * wl[n, 0]
                    tbl[:, m] = v
                    m += 1
        return tbl
    bl0b0 = bias_table(inputs['b_l0b0'], wmix_l[0], 2)   # [128,24]
    bl0b1 = bias_table(inputs['b_l0b1'], wmix_l[1], 1)   # [128,12]
    bl1b0 = bias_table(inputs['b_l1b0'], wmix_l[2], 2)   # [128,24]
    bl1b1 = bias_table(inputs['b_l1b1'], wmix_l[3], 1)   # [128,12]

    wmix = np.zeros((128, 48), dtype=f32)
    for li, wl in enumerate(wmix_l):
        for n in range(NE):
            for c in range(NC):
                wmix[:, li * 12 + n * 3 + c] = wl[n, c]

    ident = np.eye(128, dtype=f32)

    shared = {
        'Wl0': _bf16(Wl0), 'GsWg': _bf16(GsWg), 'Gq': _bf16(Gq),
        'sel16': _bf16(sel16), 'r16sel': _bf16(r16sel), 'sel12': _bf16(sel12),
        'selbc': _bf16(selbc), 'selbr': _bf16(selbr), 'oh3': _bf16(oh3),
        'Wb1': _bf16(Wb1), 'W10': _bf16(W10), 'W11': _bf16(W11),
        'gbias': gbias,
        'bl0b0': bl0b0, 'bl0b1': bl0b1, 'bl1b0': bl1b0, 'bl1b1': bl1b1,
        'wmix': wmix, 'ident': _bf16(ident),
    }
    return shared


def prep_core(inputs, r):
    """Per-core input shards (layout only)."""
    lo, hi = r * B_LOC, (r + 1) * B_LOC
    xs = inputs['sparse_embs'][lo:hi].reshape(B_LOC, KSP)      # [2048,416] f32
    xT = np.zeros((KPAD, B_LOC), dtype=ml_dtypes.bfloat16)
    xT[:KSP] = _bf16(xs.T)
    # dense features ride in the padding rows 416:429 (k-tile 3 rows 32:45)
    xT[KSP:KSP + D] = _bf16(inputs['dense_features'][lo:hi].astype(np.float32).T)
    dom = inputs['domain_ids'][lo:hi].astype(np.int64)
    dom1h = np.zeros((ND, B_LOC), dtype=ml_dtypes.bfloat16)
    for d in range(ND):
        dom1h[d] = (dom == d).astype(np.float32)
    return {'xT': xT, 'dom1h': dom1h}


def build_program(relu_dve_ok=True):
    """relu_dve_ok: biases all zero, so the relu branch may run on DVE as
    scalar_tensor_tensor (relu(p0) + tw1) with w0 pre-folded into W/bias.
    Otherwise relu runs on ACT as relu(w0*p + w0*b) via the scale port."""
    nc = bacc.Bacc(trn_type="TRN2", target_bir_lowering=False, debug=False)

    # ---- DRAM I/O ----
    t_xT = nc.dram_tensor('xT', [KPAD, B_LOC], BF16, kind="ExternalInput").ap()
    t_dom1h = nc.dram_tensor('dom1h', [ND, B_LOC], BF16, kind="ExternalInput").ap()
    t_Wl0 = nc.dram_tensor('Wl0', [4, 128, 3072], BF16, kind="ExternalInput").ap()
    t_GsWg = nc.dram_tensor('GsWg', [4, 128, 108], BF16, kind="ExternalInput").ap()
    t_Gq = nc.dram_tensor('Gq', [4, 128, 64], BF16, kind="ExternalInput").ap()
    t_sel16 = nc.dram_tensor('sel16', [16, 4], BF16, kind="ExternalInput").ap()
    t_r16sel = nc.dram_tensor('r16sel', [4, 16], BF16, kind="ExternalInput").ap()
    t_sel12 = nc.dram_tensor('sel12', [12, 36], BF16, kind="ExternalInput").ap()
    t_selbc = nc.dram_tensor('selbc', [16, 16, 128], BF16, kind="ExternalInput").ap()
    t_selbr = nc.dram_tensor('selbr', [4, 4, 128], BF16, kind="ExternalInput").ap()
    t_oh3 = nc.dram_tensor('oh3', [3, 12], BF16, kind="ExternalInput").ap()
    t_Wb1 = nc.dram_tensor('Wb1', [NE, H, 384], BF16, kind="ExternalInput").ap()
    t_W10 = nc.dram_tensor('W10', [NE, OUT, 768], BF16, kind="ExternalInput").ap()
    t_W11 = nc.dram_tensor('W11', [NE, H, 384], BF16, kind="ExternalInput").ap()
    t_gbias = nc.dram_tensor('gbias', [44, 1], F32, kind="ExternalInput").ap()
    t_bl0b0 = nc.dram_tensor('bl0b0', [128, 24], F32, kind="ExternalInput").ap()
    t_bl0b1 = nc.dram_tensor('bl0b1', [128, 12], F32, kind="ExternalInput").ap()
    t_bl1b0 = nc.dram_tensor('bl1b0', [128, 24], F32, kind="ExternalInput").ap()
    t_bl1b1 = nc.dram_tensor('bl1b1', [128, 12], F32, kind="ExternalInput").ap()
    t_wmix = nc.dram_tensor('wmix', [128, 48], F32, kind="ExternalInput").ap()
    t_ident = nc.dram_tensor('ident', [128, 128], BF16, kind="ExternalInput").ap()
    t_out = nc.dram_tensor('out', [B_LOC, OUT], F32, kind="ExternalOutput").ap()

    KT_ROWS = [128, 128, 128, 64]
    K3 = 128

    with tile.TileContext(nc) as tc:
        with (
            tc.tile_pool(name="wpool", bufs=1) as wpool,
            tc.tile_pool(name="xpool", bufs=4) as xpool,
            tc.tile_pool(name="apool", bufs=2) as apool,
            tc.tile_pool(name="hpool", bufs=2) as hpool,
            tc.tile_pool(name="bcpool", bufs=2) as bcpool,
            tc.tile_pool(name="spool", bufs=4) as spool,
            tc.tile_pool(name="opool", bufs=2) as opool,
            tc.tile_pool(name="ps_mm", bufs=2, space="PSUM") as ps_mm,
            tc.tile_pool(name="ps_sg", bufs=1, space="PSUM") as ps_sg,
            tc.tile_pool(name="ps_sm", bufs=1, space="PSUM") as ps_sm,
            tc.tile_pool(name="ps_bc", bufs=1, space="PSUM") as ps_bc,
        ):
            # ---- prologue: resident weights/constants ----
            def wtile(src_ap, shape, dtype=BF16, tag=None):
                t = wpool.tile(shape, dtype, tag=tag, name=tag)
                nc.sync.dma_start(t[:], src_ap)
                return t

            sWl0 = [wtile(t_Wl0[kt], [128, 3072], tag=f"Wl0_{kt}") for kt in range(4)]
            sGsWg = [wtile(t_GsWg[kt][:KT_ROWS[kt]], [KT_ROWS[kt], 108],
                           tag=f"GsWg{kt}") for kt in range(4)]
            sGq = [wtile(t_Gq[kt][:KT_ROWS[kt]], [KT_ROWS[kt], 64],
                         tag=f"Gq{kt}") for kt in range(4)]
            sSel = wtile(t_sel16, [16, 4], tag="sel16")
            sR16 = wtile(t_r16sel, [4, 16], tag="r16sel")
            sSel12 = wtile(t_sel12, [12, 36], tag="sel12")
            sSelBc = [wtile(t_selbc[r], [16, 128], tag=f"selbc{r}") for r in range(16)]
            sSelBr = [wtile(t_selbr[r], [4, 128], tag=f"selbr{r}") for r in range(4)]
            sOh3 = wtile(t_oh3, [3, 12], tag="oh3")
            sWb1 = [[wtile(t_Wb1[n][kt * 128:(kt + 1) * 128, :], [128, 384],
                           tag=f"Wb1_{n}{kt}") for kt in range(2)] for n in range(NE)]
            sW10 = [wtile(t_W10[n], [OUT, 768], tag=f"W10_{n}") for n in range(NE)]
            sW11 = [[wtile(t_W11[n][kt * 128:(kt + 1) * 128, :], [128, 384],
                           tag=f"W11_{n}{kt}") for kt in range(2)] for n in range(NE)]
            sGb = wtile(t_gbias, [44, 1], F32, tag="gbias")
            sB00 = wtile(t_bl0b0, [128, 24], F32, tag="bl0b0")
            sB01 = wtile(t_bl0b1, [128, 12], F32, tag="bl0b1")
            sB10 = wtile(t_bl1b0, [128, 24], F32, tag="bl1b0")
            sB11 = wtile(t_bl1b1, [128, 12], F32, tag="bl1b1")
            sWmix = wtile(t_wmix, [128, 48], F32, tag="wmix")
            sId = wtile(t_ident, [128, 128], tag="ident")

            # per-SUB-chunk state
            xk = [None] * NSUB
            gexp = [None] * NSUB
            oht = [None] * NSUB
            # per-PAIR-chunk state (e0n/wn are [*, NPC] pair tiles)
            e0n = [None] * NCHUNK
            wn = [None] * NCHUNK
            hA = [None] * NCHUNK
            hB = [None] * NCHUNK
            mixed = [None] * NCHUNK
            hC = [None] * NCHUNK
            h2 = [None] * NCHUNK

            import itertools
            uid = itertools.count()

            # ============ P0: loads, squares, fm, gates, exp (per sub) ============
            def phase0(sub):
                cc = sub * SBC
                xk[sub] = []
                for kt in range(4):
                    rows = 128 if kt < 3 else K3
                    t = xpool.tile([rows, SBC], BF16, tag=f"x{kt}", name=f"x{kt}_{sub}")
                    nc.sync.dma_start(t[0:KT_ROWS[kt], :],
                                      t_xT[kt * 128: kt * 128 + KT_ROWS[kt], cc:cc + SBC])
                    xk[sub].append(t)
                hyb = xk[sub][3]          # rows 0:64 from DRAM, 64:128 = fm below
                oh = xpool.tile([ND, SBC], BF16, tag="oh", name=f"oh_{sub}")
                nc.sync.dma_start(oh[:], t_dom1h[:, cc:cc + SBC])
                oht[sub] = oh

                xq = []
                for kt in range(4):
                    t = xpool.tile([KT_ROWS[kt], SBC], BF16, tag=f"xq{kt}",
                                   name=f"xq{kt}_{sub}", bufs=2)
                    src = xk[sub][kt][0:KT_ROWS[kt], :]
                    nc.vector.tensor_tensor(t[:], src, src, ALU.mult)
                    xq.append(t)

                # s (rows 0:64) | g0 (64:80) | gap | g1 (96:108)
                sg_ps = ps_sg.tile([128, SBC], F32, tag="sg", name=f"sg_{sub}")
                for kt in range(4):
                    nc.tensor.matmul(sg_ps[0:108, :], sGsWg[kt][:],
                                     xk[sub][kt][0:KT_ROWS[kt], :],
                                     start=(kt == 0), stop=(kt == 3))
                q_ps = ps_sm.tile([64, SBC], F32, tag="sm", name=f"q_{sub}")
                for kt in range(4):
                    nc.tensor.matmul(q_ps[:], sGq[kt][:], xq[kt][:],
                                     start=(kt == 0), stop=(kt == 3))
                ssq = spool.tile([64, SBC], F32, tag="ssq", name=f"ssq_{sub}", bufs=2)
                nc.scalar.activation(ssq[:], sg_ps[0:64, :], AF.Square,
                                     scale=float(np.sqrt(0.5)))
                nc.vector.tensor_tensor(hyb[64:128, :], ssq[:], q_ps[:],
                                        ALU.subtract)

                ge = spool.tile([44, SBC], BF16, tag="gexp", name=f"gexp_{sub}")
                nc.scalar.activation(ge[0:16, :], sg_ps[64:80, :], AF.Exp,
                                     bias=sGb[0:16, 0:1])
                nc.scalar.activation(ge[32:44, :], sg_ps[96:108, :], AF.Exp,
                                     bias=sGb[32:44, 0:1])
                gexp[sub] = ge

            # ============ P0b: softmax normalize chains (per sub) ============
            def phase0b(sub):
                ge = gexp[sub]
                ch, hf = divmod(sub, 2)
                if hf == 0:
                    e0n[ch] = spool.tile([16, NPC], BF16, tag="e0n", name=f"e0n_{ch}",
                                         bufs=2)
                    wn[ch] = spool.tile([NE, NPC], BF16, tag="wn", name=f"wn_{ch}",
                                        bufs=2)
                co = hf * SBC
                # layer-0 gate softmax normalization: e0n = e0 / rowsum_n
                s_ps = ps_sm.tile([4, SBC], F32, tag="sm", name=f"s0_{sub}")
                nc.tensor.matmul(s_ps[:], sSel[:], ge[0:16, :], start=True, stop=True)
                r = spool.tile([4, SBC], BF16, tag="r0", name=f"r0_{sub}", bufs=2)
                with nc.allow_low_precision("softmax recip feeds bf16 mix"):
                    nc.vector.reciprocal(r[:], s_ps[:])
                r16_ps = ps_sm.tile([16, SBC], F32, tag="sm", name=f"r16_{sub}")
                nc.tensor.matmul(r16_ps[:], sR16[:], r[:], start=True, stop=True)
                nc.vector.tensor_tensor(e0n[ch][:, co:co + SBC], ge[0:16, :],
                                        r16_ps[:], ALU.mult)

                # domain gate weights: mask by onehot, then expert-sum + select
                ohb_ps = ps_sm.tile([12, SBC], F32, tag="sm", name=f"ohb_{sub}")
                nc.tensor.matmul(ohb_ps[:], sOh3[:], oht[sub][:], start=True, stop=True)
                ws12 = spool.tile([12, SBC], BF16, tag="ws", name=f"ws_{sub}", bufs=2)
                nc.vector.tensor_tensor(ws12[:], ge[32:44, :], ohb_ps[:], ALU.mult)
                # sel12: rows 0:4 = expert sum (all ones); rows 32:36 = select e
                sw_ps = ps_sm.tile([36, SBC], F32, tag="sm", name=f"sw_{sub}")
                nc.tensor.matmul(sw_ps[:], sSel12[:], ws12[:], start=True, stop=True)
                rw = spool.tile([NE, SBC], BF16, tag="rw", name=f"rw_{sub}", bufs=2)
                with nc.allow_low_precision("domain softmax recip feeds bf16 mix"):
                    nc.vector.reciprocal(rw[:], sw_ps[0:4, :])
                nc.vector.tensor_tensor(wn[ch][:, co:co + SBC], sw_ps[32:36, :],
                                        rw[:], ALU.mult)

            def mixed_op_tail(plist, out_t, bias_cols, w_cols, tmp_tag):
                """plist = [p_relu, p_gelu, p_tanh] psum [128,NPC] tiles.
                out = relu(p0') + w1*gelu(p1+b1) + w2*tanh(p2+b2), with w0
                folded into p0' by host weight scaling (needs zero bias)."""
                b0, b1, b2 = bias_cols
                w0, w1, w2 = w_cols
                t1 = apool.tile([128, NPC], BF16, tag="tg",
                                name=f"tg_{next(uid)}", bufs=2)
                nc.scalar.activation(t1[:], plist[1][:], AF.Gelu_apprx_tanh, bias=b1)
                tw1 = apool.tile([128, NPC], BF16, tag="tm",
                                 name=f"tm_{next(uid)}", bufs=2)
                nc.vector.tensor_scalar(tw1[:], t1[:], w1, None, ALU.mult)
                if relu_dve_ok:
                    nc.vector.scalar_tensor_tensor(out_t[:], plist[0][:], 0.0,
                                                   tw1[:], ALU.max, ALU.add)
                else:
                    rl = apool.tile([128, NPC], BF16, tag="tr",
                                    name=f"tr_{next(uid)}", bufs=2)
                    # w0 is already folded into the c=0 weights and bias table
                    nc.scalar.activation(rl[:], plist[0][:], AF.Relu, bias=b0)
                    nc.vector.tensor_tensor(out_t[:], rl[:], tw1[:], ALU.add)
                t2 = apool.tile([128, NPC], BF16, tag="tt",
                                name=f"tt_{next(uid)}", bufs=2)
                nc.scalar.activation(t2[:], plist[2][:], AF.Tanh, bias=b2)
                tw2 = apool.tile([128, NPC], BF16, tag="tn",
                                 name=f"tn_{next(uid)}", bufs=2)
                nc.gpsimd.tensor_scalar(tw2[:], t2[:], w2, None, ALU.mult)
                nc.vector.tensor_tensor(out_t[:], out_t[:], tw2[:], ALU.add)

            # ============ P1: L0b0 -> hA ; L0b1 -> hB (per pair-chunk) ============
            def phase1(ch):
                hA[ch] = {}
                for n in range(NE):
                    if n == 1:
                        phase0b(2 * ch)
                    if n == 2:
                        phase0b(2 * ch + 1)
                    for hh in range(2):
                        out_t = hpool.tile([128, NPC], BF16, tag=f"hA{n}{hh}",
                                           name=f"hA{n}{hh}_{ch}", bufs=1)
                        hA[ch][(n, hh)] = out_t
                        ps = []
                        # emission order c=1 (gelu) first, then c=0, c=2:
                        # p1 frees right after the ACT, p0 after the STT.
                        for c in (1, 0, 2):
                            m = n * 6 + c * 2 + hh
                            p = ps_mm.tile([128, NPC], F32, tag="pmm",
                                           name=f"pA{m}_{ch}")
                            for hf in range(2):
                                sub = 2 * ch + hf
                                co = hf * SBC
                                for kt in range(3):
                                    nc.tensor.matmul(
                                        p[:, co:co + SBC],
                                        sWl0[kt][:, m * 128:(m + 1) * 128],
                                        xk[sub][kt][:], start=(kt == 0), stop=False)
                                nc.tensor.matmul(
                                    p[:, co:co + SBC],
                                    sWl0[3][0:K3, m * 128:(m + 1) * 128],
                                    xk[sub][3][:], start=False, stop=True)
                            ps.append(p)
                        ps = [ps[1], ps[0], ps[2]]   # reorder to [c0, c1, c2]
                        mcols = [n * 6 + c * 2 + hh for c in range(NC)]
                        mixed_op_tail(
                            ps, out_t,
                            [sB00[:, m:m + 1] for m in mcols],
                            [sWmix[:, n * 3 + c:n * 3 + c + 1] for c in range(NC)],
                            tmp_tag=f"A{hh}")
                hB[ch] = {}
                for n in range(NE):
                    hb = hpool.tile([128, NPC], BF16, tag=f"hB{n}", name=f"hB{n}_{ch}")
                    hB[ch][n] = hb
                    ps = []
                    for c in (1, 0, 2):
                        p = ps_mm.tile([128, NPC], F32, tag="pmm", name=f"pB{n}{c}_{ch}")
                        for hf in range(2):
                            co = hf * SBC
                            for kt in range(2):
                                nc.tensor.matmul(
                                    p[:, co:co + SBC],
                                    sWb1[n][kt][:, c * 128:(c + 1) * 128],
                                    hA[ch][(n, kt)][:, co:co + SBC],
                                    start=(kt == 0), stop=(kt == 1))
                        ps.append(p)
                    ps = [ps[1], ps[0], ps[2]]
                    mcols = [n * 3 + c for c in range(NC)]
                    mixed_op_tail(
                        ps, hb,
                        [sB01[:, m:m + 1] for m in mcols],
                        [sWmix[:, 12 + m:12 + m + 1] for m in mcols],
                        tmp_tag="B")

            # ============ P2: expert mixing (PE row-bcast + DVE/gp mults) ============
            def phase2(ch):
                mixed[ch] = {}
                for n in range(NE):
                    bcv = []
                    for e in range(NE):
                        bp = ps_bc.tile([128, NPC], F32, tag="bcp",
                                        name=f"bc{n}{e}_{ch}")
                        for hf in range(2):
                            co = hf * SBC
                            nc.tensor.matmul(bp[:, co:co + SBC], sSelBc[e * 4 + n][:],
                                             e0n[ch][:, co:co + SBC],
                                             start=True, stop=True)
                        bcv.append(bp)
                    acc = hpool.tile([128, NPC], BF16, tag=f"mix{n}",
                                     name=f"mix{n}_{ch}", bufs=1)
                    nc.vector.tensor_tensor(acc[:], hB[ch][0][:], bcv[0][:], ALU.mult)
                    for e in range(1, NE):
                        t2 = bcpool.tile([128, NPC], BF16, tag="mixt",
                                         name=f"mixt{n}{e}_{ch}")
                        nc.vector.tensor_tensor(t2[:], hB[ch][e][:], bcv[e][:],
                                                ALU.mult)
                        nc.gpsimd.tensor_tensor(acc[:], acc[:], t2[:], ALU.add)
                    mixed[ch][n] = acc

            # ============ P3: L1b0 -> hC ; L1b1 -> h2 ============
            def phase3(ch):
                hC[ch] = {}
                for n in range(NE):
                    for hh in range(2):
                        out_t = hpool.tile([128, NPC], BF16, tag=f"hC{n}{hh}",
                                           name=f"hC{n}{hh}_{ch}", bufs=1)
                        hC[ch][(n, hh)] = out_t
                        ps = []
                        for c in (1, 0, 2):
                            mt = c * 2 + hh
                            p = ps_mm.tile([128, NPC], F32, tag="pmm",
                                           name=f"pC{n}{c}{hh}_{ch}")
                            for hf in range(2):
                                co = hf * SBC
                                nc.tensor.matmul(
                                    p[:, co:co + SBC],
                                    sW10[n][:, mt * 128:(mt + 1) * 128],
                                    mixed[ch][n][:, co:co + SBC],
                                    start=True, stop=True)
                            ps.append(p)
                        ps = [ps[1], ps[0], ps[2]]
                        mcols = [n * 6 + c * 2 + hh for c in range(NC)]
                        mixed_op_tail(
                            ps, out_t,
                            [sB10[:, m:m + 1] for m in mcols],
                            [sWmix[:, 24 + n * 3 + c:24 + n * 3 + c + 1]
                             for c in range(NC)],
                            tmp_tag=f"C{hh}")
                h2[ch] = {}
                for n in range(NE):
                    hb = hpool.tile([128, NPC], BF16, tag=f"h2{n}", name=f"h2{n}_{ch}",
                                    bufs=1)
                    h2[ch][n] = hb
                    ps = []
                    for c in (1, 0, 2):
                        p = ps_mm.tile([128, NPC], F32, tag="pmm", name=f"pD{n}{c}_{ch}")
                        for hf in range(2):
                            co = hf * SBC
                            for kt in range(2):
                                nc.tensor.matmul(
                                    p[:, co:co + SBC],
                                    sW11[n][kt][:, c * 128:(c + 1) * 128],
                                    hC[ch][(n, kt)][:, co:co + SBC],
                                    start=(kt == 0), stop=(kt == 1))
                        ps.append(p)
                    ps = [ps[1], ps[0], ps[2]]
                    mcols = [n * 3 + c for c in range(NC)]
                    mixed_op_tail(
                        ps, hb,
                        [sB11[:, m:m + 1] for m in mcols],
                        [sWmix[:, 36 + m:36 + m + 1] for m in mcols],
                        tmp_tag="Dx")

            # ============ P4: domain mix (expert-major) + transpose + out ============
            def phase4(ch):
                cc = ch * NPC
                em = opool.tile([128, NPC], BF16, tag="em", name=f"em_{ch}")
                wb = []
                for e in range(2):
                    bp = ps_bc.tile([128, NPC], F32, tag="bcp", name=f"wb{e}_{ch}")
                    for hf in range(2):
                        co = hf * SBC
                        nc.tensor.matmul(bp[:, co:co + SBC], sSelBr[e][:],
                                         wn[ch][:, co:co + SBC],
                                         start=True, stop=True)
                    wb.append(bp)
                nc.vector.tensor_tensor(em[:], h2[ch][0][:], wb[0][:], ALU.mult)
                for e in range(1, NE):
                    if e + 1 < NE:
                        bp = ps_bc.tile([128, NPC], F32, tag="bcp",
                                        name=f"wb{e + 1}_{ch}")
                        for hf in range(2):
                            co = hf * SBC
                            nc.tensor.matmul(bp[:, co:co + SBC], sSelBr[e + 1][:],
                                             wn[ch][:, co:co + SBC],
                                             start=True, stop=True)
                        wb.append(bp)
                    t2 = opool.tile([128, NPC], BF16, tag="emt", name=f"emt{e}_{ch}")
                    nc.vector.tensor_tensor(t2[:], h2[ch][e][:], wb[e][:], ALU.mult)
                    nc.gpsimd.tensor_tensor(em[:], em[:], t2[:], ALU.add)
                tp = ps_sm.tile([128, NPC], BF16, tag="sm", name=f"otp_{ch}")
                for bt in range(8):
                    nc.tensor.transpose(tp[:, bt * 128:(bt + 1) * 128],
                                        em[:, bt * 128:(bt + 1) * 128], sId[:])
                ote = opool.tile([128, NPC], F32, tag="ote", name=f"ote_{ch}")
                nc.scalar.copy(ote[:], tp[:])
                for bt in range(8):
                    nc.sync.dma_start(t_out[cc + bt * 128: cc + (bt + 1) * 128, :],
                                      ote[:, bt * 128:(bt + 1) * 128])

            # ---- emission: P0 all subs, then lag-pipelined pair rounds ----
            for sub in range(NSUB):
                phase0(sub)
            phase1(0)
            phase2(0)
            phase3(0)
            phase1(1)
            phase4(0)
            phase2(1)
            phase3(1)
            phase4(1)
    nc.compile()
    return nc


_CACHE = {}


def kernel(**inputs):
    shared = prep_shared(inputs)
    in_maps = []
    for r in range(N_CORES):
        m = dict(shared)
        m.update(prep_core(inputs, r))
        in_maps.append(m)
    relu_dve_ok = (np.abs(inputs['b_l0b0']).max() == 0.0
                   and np.abs(inputs['b_l1b0']).max() == 0.0
                   and np.abs(inputs['b_l0b1']).max() == 0.0
                   and np.abs(inputs['b_l1b1']).max() == 0.0)
    key = ('nc', bool(relu_dve_ok))
    if key not in _CACHE:
        _CACHE[key] = build_program(relu_dve_ok=relu_dve_ok)
        _CACHE['nc'] = _CACHE[key]
    nc = _CACHE[key]
    res = run_bass_kernel_spmd(nc, in_maps, core_ids=list(range(N_CORES)))
    out = np.concatenate([res.results[r]['out'] for r in range(N_CORES)], axis=0)
    return out.astype(np.float32)


# revision 64
# speedup vs baseline: 2.3784x; 2.3784x over previous
"""AutoMTLSuperNet (moe_routing) Trainium2 kernel.

Batch data-parallel over 8 NeuronCores (2048 samples each, params
replicated). On-chip layout is output-channel-major ([oc, batch]); all
matmuls bf16 with f32 PSUM accumulation.

v4: batch processed as 2 pair-chunks of 1024 columns (elementwise/ACT ops
run on [*,1024] tiles to amortize per-op overhead; matmuls still stream
512-col halves into [128,1024] psum tiles). The relu candidate weight is
folded into the W_c0 matmul weights on host so the relu branch fuses into
one scalar_tensor_tensor (relu + add). Tanh-branch scale-mults run on the
otherwise idle GpSimd engine. Gate softmax normalization is folded into
the broadcast rows (no per-n reciprocal multiply). Final domain mix is
expert-major with PE row-broadcasts, then transposed once.
"""

import numpy as np
import ml_dtypes

import concourse.bass as bass
import concourse.bacc as bacc
import concourse.mybir as mybir
import concourse.tile as tile
from concourse.bass_utils import run_bass_kernel_spmd

# ---- problem dims (hardcoded per contract) ----
B, F, E, D = 16384, 26, 16, 13
NE, ND, NC = 4, 3, 3
GIN = E * (F + 1) + D            # 445
H, OUT = 256, 128
N_CORES = 8
B_LOC = B // N_CORES             # 2048
SBC = 512                        # matmul / phase0 sub-chunk columns
NPC = 1024                       # pair-chunk columns for elementwise ops
NCHUNK = B_LOC // NPC            # 2 pair-chunks
NSUB = B_LOC // SBC              # 4 sub-chunks
KSP = F * E                      # 416 flattened sparse dim
KPAD = 448
BF16 = mybir.dt.bfloat16
F32 = mybir.dt.float32

AF = mybir.ActivationFunctionType
ALU = mybir.AluOpType


def _bf16(x):
    return np.asarray(x, dtype=ml_dtypes.bfloat16)


def _softmax_np(a):
    a = np.asarray(a, dtype=np.float64)
    m = a.max(axis=-1, keepdims=True)
    e = np.exp(a - m)
    return (e / e.sum(axis=-1, keepdims=True)).astype(np.float32)


def prep_shared(inputs):
    """Host prep of all parameter tensors (input-layout + parameter-only math)."""
    f32 = np.float32
    gate_w = 1.0 / (1.0 + np.exp(-inputs['feat_alpha'].astype(np.float64)))  # [NE,F]
    gate_w = gate_w.astype(f32)

    W_l0b0 = inputs['W_l0b0'].astype(f32)   # [NE,NC,GIN,H]
    W_l0b1 = inputs['W_l0b1'].astype(f32)   # [NE,NC,H,OUT]
    W_l1b0 = inputs['W_l1b0'].astype(f32)   # [NE,NC,OUT,H]
    W_l1b1 = inputs['W_l1b1'].astype(f32)   # [NE,NC,H,OUT]

    # candidate softmax weights per mixed-op layer: [4][NE,NC]
    wmix_l = [_softmax_np(inputs[k]) for k in ('a_l0b0', 'a_l0b1', 'a_l1b0', 'a_l1b1')]

    # fold the c=0 (relu) candidate weight into the matmul weights: since
    # w0>0, w0*relu(p+b) = relu(w0*p + w0*b). Bias tables scale c=0 rows too.
    W_l0b0 = W_l0b0.copy()
    W_l0b1 = W_l0b1.copy()
    W_l1b0 = W_l1b0.copy()
    W_l1b1 = W_l1b1.copy()
    for n in range(NE):
        W_l0b0[n, 0] *= wmix_l[0][n, 0]
        W_l0b1[n, 0] *= wmix_l[1][n, 0]
        W_l1b0[n, 0] *= wmix_l[2][n, 0]
        W_l1b1[n, 0] *= wmix_l[3][n, 0]

    # ---- Wl0: lhsT ktiles [4,128, 3072]; col = n*768 + c*256 + h ----
    Wl0 = np.zeros((4, 128, NE * NC * H), dtype=f32)
    Wsp = np.zeros((KSP, NE, NC, H), dtype=f32)
    for n in range(NE):
        gvec = np.repeat(gate_w[n], E)                      # [416]
        Wsp[:, n] = W_l0b0[n, :, :KSP, :].transpose(1, 0, 2) * gvec[:, None, None]
    Wsp = Wsp.reshape(KSP, NE * NC * H)
    for kt in range(3):
        Wl0[kt, :, :] = Wsp[kt * 128:(kt + 1) * 128]
    # kt3 rows: [0:32]=sparse 384..415, [32:45]=dense, [45:64]=0, [64:128]=fm
    Wl0[3, 0:32, :] = Wsp[384:416]
    for d in range(D):
        Wl0[3, 32 + d, :] = W_l0b0[:, :, KSP + E + d, :].reshape(-1)
    for n in range(NE):
        for e in range(E):
            Wl0[3, 64 + n * 16 + e, n * 768:(n + 1) * 768] = \
                W_l0b0[n, :, KSP + e, :].reshape(768)

    # ---- GsWg: [4,128,108]; 0:64 Gs (n*16+e), 64:80 g0 (e*4+n), 96:108 g1 ----
    GsWg = np.zeros((4, 128, 108), dtype=f32)
    Gq = np.zeros((4, 128, 64), dtype=f32)
    for fe in range(KSP):
        kt, i = divmod(fe, 128)
        f_, e_ = divmod(fe, E)
        for n in range(NE):
            g = gate_w[n, f_]
            GsWg[kt, i, n * 16 + e_] = g
            Gq[kt, i, n * 16 + e_] = 0.5 * g * g   # 0.5 pre-folded
    Wg0, Wg1 = inputs['Wg0'].astype(f32), inputs['Wg1'].astype(f32)
    for i in range(KSP):
        kt, r = divmod(i, 128)
        for n in range(NE):
            for e in range(NE):
                GsWg[kt, r, 64 + e * 4 + n] = Wg0[n, i, e]
        for d in range(ND):
            for e in range(NE):
                GsWg[kt, r, 96 + d * 4 + e] = Wg1[d, i, e]
    gbias = np.zeros((44, 1), dtype=f32)
    for n in range(NE):
        for e in range(NE):
            gbias[e * 4 + n, 0] = inputs['bg0'][n, e] + inputs['beta0'][n, e]
    for d in range(ND):
        for e in range(NE):
            gbias[32 + d * 4 + e, 0] = inputs['bg1'][d, e] + inputs['beta1'][d, e]
    # sel16 [16,4]: row e*4+n -> col n  (row sums per layer-0 gate n)
    sel16 = np.zeros((16, 4), dtype=f32)
    for e in range(NE):
        for n in range(NE):
            sel16[e * 4 + n, n] = 1.0
    # r16sel [4,16]: broadcast r0 row n to rows e*4+n
    r16sel = np.zeros((4, 16), dtype=f32)
    for e in range(NE):
        for n in range(NE):
            r16sel[n, e * 4 + n] = 1.0
    # selbc/selbr: lhsT picks one row of rhs, broadcast to 128 partitions
    selbc = np.zeros((16, 16, 128), dtype=f32)
    for r_ in range(16):
        selbc[r_, r_, :] = 1.0
    selbr = np.zeros((4, 4, 128), dtype=f32)
    for r_ in range(4):
        selbr[r_, r_, :] = 1.0
    # oh3 [3,12]: broadcast domain-onehot row d to rows 4d..4d+3
    oh3 = np.zeros((3, 12), dtype=f32)
    for d in range(ND):
        oh3[d, 4 * d:4 * d + 4] = 1.0
    # sel12 [12,36]: cols 0:4 = all-ones (expert sum); cols 32:36 pick expert e
    sel12 = np.zeros((12, 36), dtype=f32)
    sel12[:, 0:4] = 1.0
    for d in range(ND):
        for e in range(NE):
            sel12[4 * d + e, 32 + e] = 1.0

    # ---- later layer weights ----
    Wb1 = np.zeros((NE, H, NC * OUT), dtype=f32)       # lhsT col = c*128+o
    for n in range(NE):
        Wb1[n] = W_l0b1[n].transpose(1, 0, 2).reshape(H, NC * OUT)
    W10 = np.zeros((NE, OUT, NC * H), dtype=f32)       # col = c*256+h
    for n in range(NE):
        W10[n] = W_l1b0[n].transpose(1, 0, 2).reshape(OUT, NC * H)
    W11 = np.zeros((NE, H, NC * OUT), dtype=f32)
    for n in range(NE):
        W11[n] = W_l1b1[n].transpose(1, 0, 2).reshape(H, NC * OUT)

    # ---- bias column tables (per-partition vectors), w-scaled for relu c=0 ----
    def bias_table(bmat, wl, n_mt):
        Wd = bmat.shape[-1]
        tbl = np.zeros((128, NE * NC * (Wd // 128)), dtype=f32)
        m = 0
        for n in range(NE):
            for c in range(NC):
                for hh in range(Wd // 128):
                    v = bmat[n, c, hh * 128:(hh + 1) * 128].astype(f32)
                    if c == 0:
                        v = v * wl[n, 0]
                    tbl[:, m] = v
                    m += 1
        return tbl
    bl0b0 = bias_table(inputs['b_l0b0'], wmix_l[0], 2)   # [128,24]
    bl0b1 = bias_table(inputs['b_l0b1'], wmix_l[1], 1)   # [128,12]
    bl1b0 = bias_table(inputs['b_l1b0'], wmix_l[2], 2)   # [128,24]
    bl1b1 = bias_table(inputs['b_l1b1'], wmix_l[3], 1)   # [128,12]

    wmix = np.zeros((128, 48), dtype=f32)
    for li, wl in enumerate(wmix_l):
        for n in range(NE):
            for c in range(NC):
                wmix[:, li * 12 + n * 3 + c] = wl[n, c]

    ident = np.eye(128, dtype=f32)

    shared = {
        'Wl0': _bf16(Wl0), 'GsWg': _bf16(GsWg), 'Gq': _bf16(Gq),
        'sel16': _bf16(sel16), 'r16sel': _bf16(r16sel), 'sel12': _bf16(sel12),
        'selbc': _bf16(selbc), 'selbr': _bf16(selbr), 'oh3': _bf16(oh3),
        'Wb1': _bf16(Wb1), 'W10': _bf16(W10), 'W11': _bf16(W11),
        'gbias': gbias,
        'bl0b0': bl0b0, 'bl0b1': bl0b1, 'bl1b0': bl1b0, 'bl1b1': bl1b1,
        'wmix': wmix, 'ident': _bf16(ident),
    }
    return shared


def prep_core(inputs, r):
    """Per-core input shards (layout only)."""
    lo, hi = r * B_LOC, (r + 1) * B_LOC
    xs = inputs['sparse_embs'][lo:hi].reshape(B_LOC, KSP)      # [2048,416] f32
    xT = np.zeros((KPAD, B_LOC), dtype=ml_dtypes.bfloat16)
    xT[:KSP] = _bf16(xs.T)
    # dense features ride in the padding rows 416:429 (k-tile 3 rows 32:45)
    xT[KSP:KSP + D] = _bf16(inputs['dense_features'][lo:hi].astype(np.float32).T)
    dom = inputs['domain_ids'][lo:hi].astype(np.int64)
    dom1h = np.zeros((ND, B_LOC), dtype=ml_dtypes.bfloat16)
    for d in range(ND):
        dom1h[d] = (dom == d).astype(np.float32)
    return {'xT': xT, 'dom1h': dom1h}


def build_program(relu_dve_ok=True):
    """relu_dve_ok: biases all zero, so the relu branch may run on DVE as
    scalar_tensor_tensor (relu(p0) + tw1) with w0 pre-folded into W/bias.
    Otherwise relu runs on ACT as relu(w0*p + w0*b) via the scale port."""
    nc = bacc.Bacc(trn_type="TRN2", target_bir_lowering=False, debug=False)

    # ---- DRAM I/O ----
    t_xT = nc.dram_tensor('xT', [KPAD, B_LOC], BF16, kind="ExternalInput").ap()
    t_dom1h = nc.dram_tensor('dom1h', [ND, B_LOC], BF16, kind="ExternalInput").ap()
    t_Wl0 = nc.dram_tensor('Wl0', [4, 128, 3072], BF16, kind="ExternalInput").ap()
    t_GsWg = nc.dram_tensor('GsWg', [4, 128, 108], BF16, kind="ExternalInput").ap()
    t_Gq = nc.dram_tensor('Gq', [4, 128, 64], BF16, kind="ExternalInput").ap()
    t_sel16 = nc.dram_tensor('sel16', [16, 4], BF16, kind="ExternalInput").ap()
    t_r16sel = nc.dram_tensor('r16sel', [4, 16], BF16, kind="ExternalInput").ap()
    t_sel12 = nc.dram_tensor('sel12', [12, 36], BF16, kind="ExternalInput").ap()
    t_selbc = nc.dram_tensor('selbc', [16, 16, 128], BF16, kind="ExternalInput").ap()
    t_selbr = nc.dram_tensor('selbr', [4, 4, 128], BF16, kind="ExternalInput").ap()
    t_oh3 = nc.dram_tensor('oh3', [3, 12], BF16, kind="ExternalInput").ap()
    t_Wb1 = nc.dram_tensor('Wb1', [NE, H, 384], BF16, kind="ExternalInput").ap()
    t_W10 = nc.dram_tensor('W10', [NE, OUT, 768], BF16, kind="ExternalInput").ap()
    t_W11 = nc.dram_tensor('W11', [NE, H, 384], BF16, kind="ExternalInput").ap()
    t_gbias = nc.dram_tensor('gbias', [44, 1], F32, kind="ExternalInput").ap()
    t_bl0b0 = nc.dram_tensor('bl0b0', [128, 24], F32, kind="ExternalInput").ap()
    t_bl0b1 = nc.dram_tensor('bl0b1', [128, 12], F32, kind="ExternalInput").ap()
    t_bl1b0 = nc.dram_tensor('bl1b0', [128, 24], F32, kind="ExternalInput").ap()
    t_bl1b1 = nc.dram_tensor('bl1b1', [128, 12], F32, kind="ExternalInput").ap()
    t_wmix = nc.dram_tensor('wmix', [128, 48], F32, kind="ExternalInput").ap()
    t_ident = nc.dram_tensor('ident', [128, 128], BF16, kind="ExternalInput").ap()
    t_out = nc.dram_tensor('out', [B_LOC, OUT], F32, kind="ExternalOutput").ap()

    KT_ROWS = [128, 128, 128, 64]
    K3 = 128

    with tile.TileContext(nc) as tc:
        with (
            tc.tile_pool(name="wpool", bufs=1) as wpool,
            tc.tile_pool(name="xpool", bufs=4) as xpool,
            tc.tile_pool(name="apool", bufs=2) as apool,
            tc.tile_pool(name="hpool", bufs=2) as hpool,
            tc.tile_pool(name="bcpool", bufs=2) as bcpool,
            tc.tile_pool(name="spool", bufs=4) as spool,
            tc.tile_pool(name="opool", bufs=2) as opool,
            tc.tile_pool(name="ps_mm", bufs=2, space="PSUM") as ps_mm,
            tc.tile_pool(name="ps_sg", bufs=1, space="PSUM") as ps_sg,
            tc.tile_pool(name="ps_sm", bufs=1, space="PSUM") as ps_sm,
            tc.tile_pool(name="ps_bc", bufs=1, space="PSUM") as ps_bc,
        ):
            # ---- prologue: resident weights/constants ----
            def wtile(src_ap, shape, dtype=BF16, tag=None):
                t = wpool.tile(shape, dtype, tag=tag, name=tag)
                nc.sync.dma_start(t[:], src_ap)
                return t

            sWl0 = [wtile(t_Wl0[kt], [128, 3072], tag=f"Wl0_{kt}") for kt in range(4)]
            sGsWg = [wtile(t_GsWg[kt][:KT_ROWS[kt]], [KT_ROWS[kt], 108],
                           tag=f"GsWg{kt}") for kt in range(4)]
            sGq = [wtile(t_Gq[kt][:KT_ROWS[kt]], [KT_ROWS[kt], 64],
                         tag=f"Gq{kt}") for kt in range(4)]
            sSel = wtile(t_sel16, [16, 4], tag="sel16")
            sR16 = wtile(t_r16sel, [4, 16], tag="r16sel")
            sSel12 = wtile(t_sel12, [12, 36], tag="sel12")
            sSelBc = [wtile(t_selbc[r], [16, 128], tag=f"selbc{r}") for r in range(16)]
            sSelBr = [wtile(t_selbr[r], [4, 128], tag=f"selbr{r}") for r in range(4)]
            sOh3 = wtile(t_oh3, [3, 12], tag="oh3")
            sWb1 = [[wtile(t_Wb1[n][kt * 128:(kt + 1) * 128, :], [128, 384],
                           tag=f"Wb1_{n}{kt}") for kt in range(2)] for n in range(NE)]
            sW10 = [wtile(t_W10[n], [OUT, 768], tag=f"W10_{n}") for n in range(NE)]
            sW11 = [[wtile(t_W11[n][kt * 128:(kt + 1) * 128, :], [128, 384],
                           tag=f"W11_{n}{kt}") for kt in range(2)] for n in range(NE)]
            sGb = wtile(t_gbias, [44, 1], F32, tag="gbias")
            sB00 = wtile(t_bl0b0, [128, 24], F32, tag="bl0b0")
            sB01 = wtile(t_bl0b1, [128, 12], F32, tag="bl0b1")
            sB10 = wtile(t_bl1b0, [128, 24], F32, tag="bl1b0")
            sB11 = wtile(t_bl1b1, [128, 12], F32, tag="bl1b1")
            sWmix = wtile(t_wmix, [128, 48], F32, tag="wmix")
            sId = wtile(t_ident, [128, 128], tag="ident")

            # per-SUB-chunk state
            xk = [None] * NSUB
            gexp = [None] * NSUB
            oht = [None] * NSUB
            # per-PAIR-chunk state (e0n/wn are [*, NPC] pair tiles)
            e0n = [None] * NCHUNK
            wn = [None] * NCHUNK
            hA = [None] * NCHUNK
            hB = [None] * NCHUNK
            mixed = [None] * NCHUNK
            hC = [None] * NCHUNK
            h2 = [None] * NCHUNK

            import itertools
            uid = itertools.count()

            # ============ P0: loads, squares, fm, gates, exp (per sub) ============
            def phase0(sub):
                cc = sub * SBC
                xk[sub] = []
                for kt in range(4):
                    rows = 128 if kt < 3 else K3
                    t = xpool.tile([rows, SBC], BF16, tag=f"x{kt}", name=f"x{kt}_{sub}")
                    nc.sync.dma_start(t[0:KT_ROWS[kt], :],
                                      t_xT[kt * 128: kt * 128 + KT_ROWS[kt], cc:cc + SBC])
                    xk[sub].append(t)
                hyb = xk[sub][3]          # rows 0:64 from DRAM, 64:128 = fm below
                oh = xpool.tile([ND, SBC], BF16, tag="oh", name=f"oh_{sub}")
                nc.sync.dma_start(oh[:], t_dom1h[:, cc:cc + SBC])
                oht[sub] = oh

                xq = []
                for kt in range(4):
                    t = xpool.tile([KT_ROWS[kt], SBC], BF16, tag=f"xq{kt}",
                                   name=f"xq{kt}_{sub}", bufs=2)
                    src = xk[sub][kt][0:KT_ROWS[kt], :]
                    nc.gpsimd.tensor_tensor(t[:], src, src, ALU.mult)
                    xq.append(t)

                # s (rows 0:64) | g0 (64:80) | gap | g1 (96:108)
                sg_ps = ps_sg.tile([128, SBC], F32, tag="sg", name=f"sg_{sub}")
                for kt in range(4):
                    nc.tensor.matmul(sg_ps[0:108, :], sGsWg[kt][:],
                                     xk[sub][kt][0:KT_ROWS[kt], :],
                                     start=(kt == 0), stop=(kt == 3))
                q_ps = ps_sm.tile([64, SBC], F32, tag="sm", name=f"q_{sub}")
                for kt in range(4):
                    nc.tensor.matmul(q_ps[:], sGq[kt][:], xq[kt][:],
                                     start=(kt == 0), stop=(kt == 3))
                ssq = spool.tile([64, SBC], F32, tag="ssq", name=f"ssq_{sub}", bufs=2)
                nc.scalar.activation(ssq[:], sg_ps[0:64, :], AF.Square,
                                     scale=float(np.sqrt(0.5)))
                nc.vector.tensor_tensor(hyb[64:128, :], ssq[:], q_ps[:],
                                        ALU.subtract)

                ge = spool.tile([44, SBC], BF16, tag="gexp", name=f"gexp_{sub}")
                nc.scalar.activation(ge[0:16, :], sg_ps[64:80, :], AF.Exp,
                                     bias=sGb[0:16, 0:1])
                nc.scalar.activation(ge[32:44, :], sg_ps[96:108, :], AF.Exp,
                                     bias=sGb[32:44, 0:1])
                gexp[sub] = ge

            # ============ P0b: softmax normalize chains (per sub) ============
            def phase0b(sub):
                ge = gexp[sub]
                ch, hf = divmod(sub, 2)
                if hf == 0:
                    e0n[ch] = spool.tile([16, NPC], BF16, tag="e0n", name=f"e0n_{ch}",
                                         bufs=2)
                    wn[ch] = spool.tile([NE, NPC], BF16, tag="wn", name=f"wn_{ch}",
                                        bufs=2)
                co = hf * SBC
                # layer-0 gate softmax normalization: e0n = e0 / rowsum_n
                s_ps = ps_sm.tile([4, SBC], F32, tag="sm", name=f"s0_{sub}")
                nc.tensor.matmul(s_ps[:], sSel[:], ge[0:16, :], start=True, stop=True)
                rf = spool.tile([4, SBC], F32, tag="r0f", name=f"r0f_{sub}", bufs=2)
                nc.vector.reciprocal_approx_fast(rf[:], s_ps[:])
                r = spool.tile([4, SBC], BF16, tag="r0", name=f"r0_{sub}", bufs=2)
                nc.vector.tensor_scalar(r[:], rf[:], 1.0, None, ALU.mult)
                r16_ps = ps_sm.tile([16, SBC], F32, tag="sm", name=f"r16_{sub}")
                nc.tensor.matmul(r16_ps[:], sR16[:], r[:], start=True, stop=True)
                nc.vector.tensor_tensor(e0n[ch][:, co:co + SBC], ge[0:16, :],
                                        r16_ps[:], ALU.mult)

                # domain gate weights: mask by onehot, then expert-sum + select
                ohb_ps = ps_sm.tile([12, SBC], F32, tag="sm", name=f"ohb_{sub}")
                nc.tensor.matmul(ohb_ps[:], sOh3[:], oht[sub][:], start=True, stop=True)
                ws12 = spool.tile([12, SBC], BF16, tag="ws", name=f"ws_{sub}", bufs=2)
                nc.vector.tensor_tensor(ws12[:], ge[32:44, :], ohb_ps[:], ALU.mult)
                # sel12: rows 0:4 = expert sum (all ones); rows 32:36 = select e
                sw_ps = ps_sm.tile([36, SBC], F32, tag="sm", name=f"sw_{sub}")
                nc.tensor.matmul(sw_ps[:], sSel12[:], ws12[:], start=True, stop=True)
                rw = spool.tile([NE, SBC], F32, tag="rw", name=f"rw_{sub}", bufs=2)
                nc.vector.reciprocal_approx_fast(rw[:], sw_ps[0:4, :])
                nc.vector.tensor_tensor(wn[ch][:, co:co + SBC], sw_ps[32:36, :],
                                        rw[:], ALU.mult)

            def mixed_op_tail(plist, out_t, bias_cols, w_cols, tmp_tag):
                """plist = [p_relu, p_gelu, p_tanh] psum [128,NPC] tiles.
                out = relu(p0' + b0') + w1*gelu(p1+b1) + w2*tanh(p2+b2); w0 is
                folded into the c=0 weights and bias table on host. All three
                activations run on ACT at 1024 wide; the two weighted adds are
                fused scalar_tensor_tensor ops on DVE."""
                b0, b1, b2 = bias_cols
                w0, w1, w2 = w_cols
                rl = apool.tile([128, NPC], BF16, tag="tr",
                                name=f"tr_{next(uid)}", bufs=2)
                nc.scalar.activation(rl[:], plist[0][:], AF.Relu, bias=b0)
                t1 = apool.tile([128, NPC], BF16, tag="tg",
                                name=f"tg_{next(uid)}", bufs=2)
                nc.scalar.activation(t1[:], plist[1][:], AF.Gelu_apprx_tanh, bias=b1)
                nc.vector.scalar_tensor_tensor(out_t[:], t1[:], w1, rl[:],
                                               ALU.mult, ALU.add)
                t2 = apool.tile([128, NPC], BF16, tag="tt",
                                name=f"tt_{next(uid)}", bufs=2)
                nc.scalar.activation(t2[:], plist[2][:], AF.Tanh, bias=b2)
                nc.vector.scalar_tensor_tensor(out_t[:], t2[:], w2, out_t[:],
                                               ALU.mult, ALU.add)

            # ============ P1: L0b0 -> hA ; L0b1 -> hB (per pair-chunk) ============
            def phase1(ch):
                hA[ch] = {}
                for n in range(NE):
                    if n == 1:
                        phase0b(2 * ch)
                    if n == 2:
                        phase0b(2 * ch + 1)
                    for hh in range(2):
                        out_t = hpool.tile([128, NPC], BF16, tag=f"hA{n}{hh}",
                                           name=f"hA{n}{hh}_{ch}", bufs=1)
                        hA[ch][(n, hh)] = out_t
                        ps = []
                        # emission order c=1 (gelu) first, then c=0, c=2:
                        # p1 frees right after the ACT, p0 after the STT.
                        for c in (1, 0, 2):
                            m = n * 6 + c * 2 + hh
                            p = ps_mm.tile([128, NPC], F32, tag="pmm",
                                           name=f"pA{m}_{ch}")
                            for hf in range(2):
                                sub = 2 * ch + hf
                                co = hf * SBC
                                for kt in range(3):
                                    nc.tensor.matmul(
                                        p[:, co:co + SBC],
                                        sWl0[kt][:, m * 128:(m + 1) * 128],
                                        xk[sub][kt][:], start=(kt == 0), stop=False)
                                nc.tensor.matmul(
                                    p[:, co:co + SBC],
                                    sWl0[3][0:K3, m * 128:(m + 1) * 128],
                                    xk[sub][3][:], start=False, stop=True)
                            ps.append(p)
                        ps = [ps[1], ps[0], ps[2]]   # reorder to [c0, c1, c2]
                        mcols = [n * 6 + c * 2 + hh for c in range(NC)]
                        mixed_op_tail(
                            ps, out_t,
                            [sB00[:, m:m + 1] for m in mcols],
                            [sWmix[:, n * 3 + c:n * 3 + c + 1] for c in range(NC)],
                            tmp_tag=f"A{hh}")
                hB[ch] = {}
                for n in range(NE):
                    hb = hpool.tile([128, NPC], BF16, tag=f"hB{n}", name=f"hB{n}_{ch}")
                    hB[ch][n] = hb
                    ps = []
                    for c in (1, 0, 2):
                        p = ps_mm.tile([128, NPC], F32, tag="pmm", name=f"pB{n}{c}_{ch}")
                        for hf in range(2):
                            co = hf * SBC
                            for kt in range(2):
                                nc.tensor.matmul(
                                    p[:, co:co + SBC],
                                    sWb1[n][kt][:, c * 128:(c + 1) * 128],
                                    hA[ch][(n, kt)][:, co:co + SBC],
                                    start=(kt == 0), stop=(kt == 1))
                        ps.append(p)
                    ps = [ps[1], ps[0], ps[2]]
                    mcols = [n * 3 + c for c in range(NC)]
                    mixed_op_tail(
                        ps, hb,
                        [sB01[:, m:m + 1] for m in mcols],
                        [sWmix[:, 12 + m:12 + m + 1] for m in mcols],
                        tmp_tag="B")

            # ============ P2: expert mixing (PE row-bcast + DVE/gp mults) ============
            def phase2(ch):
                mixed[ch] = {}
                for n in range(NE):
                    bcv = []
                    for e in range(NE):
                        bp = ps_bc.tile([128, NPC], F32, tag="bcp",
                                        name=f"bc{n}{e}_{ch}")
                        for hf in range(2):
                            co = hf * SBC
                            nc.tensor.matmul(bp[:, co:co + SBC], sSelBc[e * 4 + n][:],
                                             e0n[ch][:, co:co + SBC],
                                             start=True, stop=True)
                        bcv.append(bp)
                    acc = hpool.tile([128, NPC], BF16, tag=f"mix{n}",
                                     name=f"mix{n}_{ch}", bufs=1)
                    for hf in range(2):
                        co = hf * SBC
                        nc.vector.tensor_tensor(acc[:, co:co + SBC],
                                                hB[ch][0][:, co:co + SBC],
                                                bcv[0][:, co:co + SBC], ALU.mult)
                    for e in range(1, NE):
                        t2 = bcpool.tile([128, NPC], BF16, tag="mixt",
                                         name=f"mixt{n}{e}_{ch}")
                        for hf in range(2):
                            co = hf * SBC
                            nc.vector.tensor_tensor(t2[:, co:co + SBC],
                                                    hB[ch][e][:, co:co + SBC],
                                                    bcv[e][:, co:co + SBC], ALU.mult)
                        nc.gpsimd.tensor_tensor(acc[:], acc[:], t2[:], ALU.add)
                    mixed[ch][n] = acc

            # ============ P3: L1b0 -> hC ; L1b1 -> h2 ============
            def phase3(ch):
                hC[ch] = {}
                for n in range(NE):
                    for hh in range(2):
                        out_t = hpool.tile([128, NPC], BF16, tag=f"hC{n}{hh}",
                                           name=f"hC{n}{hh}_{ch}", bufs=1)
                        hC[ch][(n, hh)] = out_t
                        ps = []
                        for c in (1, 0, 2):
                            mt = c * 2 + hh
                            p = ps_mm.tile([128, NPC], F32, tag="pmm",
                                           name=f"pC{n}{c}{hh}_{ch}")
                            for hf in range(2):
                                co = hf * SBC
                                nc.tensor.matmul(
                                    p[:, co:co + SBC],
                                    sW10[n][:, mt * 128:(mt + 1) * 128],
                                    mixed[ch][n][:, co:co + SBC],
                                    start=True, stop=True)
                            ps.append(p)
                        ps = [ps[1], ps[0], ps[2]]
                        mcols = [n * 6 + c * 2 + hh for c in range(NC)]
                        mixed_op_tail(
                            ps, out_t,
                            [sB10[:, m:m + 1] for m in mcols],
                            [sWmix[:, 24 + n * 3 + c:24 + n * 3 + c + 1]
                             for c in range(NC)],
                            tmp_tag=f"C{hh}")
                h2[ch] = {}
                for n in range(NE):
                    hb = hpool.tile([128, NPC], BF16, tag=f"h2{n}", name=f"h2{n}_{ch}",
                                    bufs=1)
                    h2[ch][n] = hb
                    ps = []
                    for c in (1, 0, 2):
                        p = ps_mm.tile([128, NPC], F32, tag="pmm", name=f"pD{n}{c}_{ch}")
                        for hf in range(2):
                            co = hf * SBC
                            for kt in range(2):
                                nc.tensor.matmul(
                                    p[:, co:co + SBC],
                                    sW11[n][kt][:, c * 128:(c + 1) * 128],
                                    hC[ch][(n, kt)][:, co:co + SBC],
                                    start=(kt == 0), stop=(kt == 1))
                        ps.append(p)
                    ps = [ps[1], ps[0], ps[2]]
                    mcols = [n * 3 + c for c in range(NC)]
                    mixed_op_tail(
                        ps, hb,
                        [sB11[:, m:m + 1] for m in mcols],
                        [sWmix[:, 36 + m:36 + m + 1] for m in mcols],
                        tmp_tag="Dx")

            # ============ P4: domain mix (expert-major) + transpose + out ============
            def phase4(ch):
                cc = ch * NPC
                em = opool.tile([128, NPC], BF16, tag="em", name=f"em_{ch}")
                wb = []
                for e in range(2):
                    bp = ps_bc.tile([128, NPC], F32, tag="bcp", name=f"wb{e}_{ch}")
                    for hf in range(2):
                        co = hf * SBC
                        nc.tensor.matmul(bp[:, co:co + SBC], sSelBr[e][:],
                                         wn[ch][:, co:co + SBC],
                                         start=True, stop=True)
                    wb.append(bp)
                for hf in range(2):
                    co = hf * SBC
                    nc.vector.tensor_tensor(em[:, co:co + SBC],
                                            h2[ch][0][:, co:co + SBC],
                                            wb[0][:, co:co + SBC], ALU.mult)
                for e in range(1, NE):
                    if e + 1 < NE:
                        bp = ps_bc.tile([128, NPC], F32, tag="bcp",
                                        name=f"wb{e + 1}_{ch}")
                        for hf in range(2):
                            co = hf * SBC
                            nc.tensor.matmul(bp[:, co:co + SBC], sSelBr[e + 1][:],
                                             wn[ch][:, co:co + SBC],
                                             start=True, stop=True)
                        wb.append(bp)
                    t2 = opool.tile([128, NPC], BF16, tag="emt", name=f"emt{e}_{ch}")
                    for hf in range(2):
                        co = hf * SBC
                        nc.vector.tensor_tensor(t2[:, co:co + SBC],
                                                h2[ch][e][:, co:co + SBC],
                                                wb[e][:, co:co + SBC], ALU.mult)
                    nc.gpsimd.tensor_tensor(em[:], em[:], t2[:], ALU.add)
                tp = ps_sm.tile([128, NPC], BF16, tag="sm", name=f"otp_{ch}")
                for bt in range(8):
                    nc.tensor.transpose(tp[:, bt * 128:(bt + 1) * 128],
                                        em[:, bt * 128:(bt + 1) * 128], sId[:])
                ote = opool.tile([128, NPC], F32, tag="ote", name=f"ote_{ch}")
                nc.scalar.copy(ote[:], tp[:])
                for bt in range(8):
                    nc.sync.dma_start(t_out[cc + bt * 128: cc + (bt + 1) * 128, :],
                                      ote[:, bt * 128:(bt + 1) * 128])

            # ---- emission: P0 all subs, then lag-pipelined pair rounds ----
            for sub in range(NSUB):
                phase0(sub)
            phase1(0)
            phase2(0)
            phase3(0)
            phase1(1)
            phase4(0)
            phase2(1)
            phase3(1)
            phase4(1)
    nc.compile()
    return nc


_CACHE = {}


def kernel(**inputs):
    shared = prep_shared(inputs)
    in_maps = []
    for r in range(N_CORES):
        m = dict(shared)
        m.update(prep_core(inputs, r))
        in_maps.append(m)
    relu_dve_ok = (np.abs(inputs['b_l0b0']).max() == 0.0
                   and np.abs(inputs['b_l1b0']).max() == 0.0
                   and np.abs(inputs['b_l0b1']).max() == 0.0
                   and np.abs(inputs['b_l1b1']).max() == 0.0)
    key = ('nc', bool(relu_dve_ok))
    if key not in _CACHE:
        _CACHE[key] = build_program(relu_dve_ok=relu_dve_ok)
        _CACHE['nc'] = _CACHE[key]
    nc = _CACHE[key]
    res = run_bass_kernel_spmd(nc, in_maps, core_ids=list(range(N_CORES)))
    out = np.concatenate([res.results[r]['out'] for r in range(N_CORES)], axis=0)
    return out.astype(np.float32)
